# revision 1
# baseline (speedup 1.0000x reference)
"""Trainium2 Bass kernel for fused multi-head attention (16 heads, d=64,
b=2, n=2048, h=1024) across 8 NeuronCores.

Sharding: tensor-parallel over heads x data-parallel over batch.
Core c handles batch c//4 and heads [4*(c%4), 4*(c%4)+4). Each core
computes Q^T/K^T/V for its 4 heads over the full 2048-token sequence from
a replicated (per-batch) x — no communication before attention, so the
TensorE/ScalarE pipeline starts immediately and the kernel-start
cross-core barrier is hidden. After attention, a small bf16 AllToAll per
512-row piece (Ulysses-style) swaps head-shards for row-shards, so each
core runs the output projection locally with the full 1024 contraction —
no reduction collective and only ~1MB of total wire traffic per core.

Attention per head runs in scores-transposed layout [k, q] (softmax
without max subtraction -- logits are ~N(0,1) here), the softmax
denominator comes from a ones-column appended to V (M=65 AV matmuls), and
normalization is applied to the small attn_out^T [64, 512] tiles via a PE
broadcast of 1/denom. The attn^T layout feeds both the AllToAll and the
output projection lhsT directly, so no on-chip transposes are needed
anywhere. All matmuls bf16 with f32 PSUM accumulation; softmax exp on
ScalarE is the pace-setting engine (~147 us/core).
"""

import sys

if "/opt/trn_rl_repo" not in sys.path:
    sys.path.insert(0, "/opt/trn_rl_repo")

import numpy as np
import ml_dtypes

import concourse.bass as bass
import concourse.mybir as mybir
import concourse.tile as tile
from concourse import bacc
from concourse.bass import ts
from concourse.bass_utils import run_bass_kernel_spmd

BF16 = mybir.dt.bfloat16
F32 = mybir.dt.float32
ADD = mybir.AluOpType.add
MULT = mybir.AluOpType.mult
BYPASS = mybir.AluOpType.bypass
EXP = mybir.ActivationFunctionType.Exp

HEADS, D, H, N, B = 16, 64, 1024, 2048, 2
NC_ = 8
LH = 4            # local heads per core
LPAIRS = 2        # local head pairs
KC = 16           # k chunks of 128 over n=2048
QC = 4            # q chunks of 512 over n=2048 (= AllToAll pieces)
LVW = LH * 65     # 260: local v-aug width
LQK = LH * D      # 256 local q (or k) columns
GROUPS = [[0, 1, 2, 3], [4, 5, 6, 7]]


def build_nc():
    nc = bacc.Bacc("TRN2", target_bir_lowering=False, debug=False, num_devices=NC_)

    xT = nc.declare_dram_parameter("xT", [H, N], BF16, isOutput=False)
    wqk = nc.declare_dram_parameter("wqk", [H, 2 * LQK], BF16, isOutput=False)
    wv = nc.declare_dram_parameter("wv", [H, LVW], BF16, isOutput=False)
    wout = nc.declare_dram_parameter("wout", [H, H], BF16, isOutput=False)
    cos2 = nc.declare_dram_parameter("cos2", [128, N], BF16, isOutput=False)
    # sswp[p] = sin value read at SOURCE partition p during the shuffle:
    # p%64 < 32 -> +sin[p%64+32], else -sin[p%64-32]
    sinm = nc.declare_dram_parameter("sinm", [128, N], BF16, isOutput=False)
    # msk[:,0]=1 iff this core's batch is 0; msk[:,1]=1 iff batch 1
    msk = nc.declare_dram_parameter("msk", [128, 2], F32, isOutput=False)
    out = nc.declare_dram_parameter("out", [QC, 128, H], F32, isOutput=True)

    with tile.TileContext(nc) as tc:
        with (
            tc.tile_pool(name="dram", bufs=1, space="DRAM") as dram,
            tc.tile_pool(name="sb", bufs=1) as sb,
            tc.tile_pool(name="sbw", bufs=1) as sbw,
            tc.tile_pool(name="psum", bufs=2, space="PSUM") as ps,
        ):
            a2a_in = [dram.tile([8, 2 * 128, 128], BF16, name=f"ain{i}")
                      for i in range(QC)]
            a2a_out = [dram.tile([8, 2 * 128, 128], BF16, name=f"aout{i}")
                       for i in range(QC)]

            warm_in = dram.tile([8, 128], BF16, name="warm_in")
            warm_out = dram.tile([8, 128], BF16, name="warm_out")
            warm_sb = sbw.tile([1, 128], BF16)
            nc.vector.memset(warm_sb[:, :], 0.0)
            nc.scalar.dma_start(warm_in[0:1, :], warm_sb[:, :])
            nc.gpsimd.collective_compute(
                "AllToAll", BYPASS, replica_groups=[list(range(8))],
                ins=[warm_in.opt()], outs=[warm_out.opt()])

            # ---- stage inputs; x split across three DMA queues ----
            xt_sb = sbw.tile([128, 8 * N], BF16)
            wqk_sb = sbw.tile([128, 8 * 2 * LQK], BF16)
            wv_sb = sbw.tile([128, 8 * LVW], BF16)
            wout_sb = sbw.tile([128, 8 * H], BF16)
            cos2_sb = sbw.tile([128, N], BF16)
            sinm_sb = sbw.tile([128, N], BF16)
            ones_sb = sbw.tile([1, D], BF16)
            msk_sb = sbw.tile([128, 2], F32)
            nc.sync.dma_start(msk_sb[:, :], msk[:, :])
            for hk in range(8):
                nc.sync.dma_start(wqk_sb[:, ts(hk, 2 * LQK)], wqk[ts(hk, 128), :])
            for hk in range(8):
                nc.sync.dma_start(wv_sb[:, ts(hk, LVW)], wv[ts(hk, 128), :])
            engs = [nc.sync, nc.gpsimd, nc.scalar]
            for hk in range(8):
                engs[hk % 3].dma_start(xt_sb[:, ts(hk, N)], xT[ts(hk, 128), :])
            nc.gpsimd.dma_start(cos2_sb[:, :], cos2[:, :])
            nc.gpsimd.dma_start(sinm_sb[:, :], sinm[:, :])
            for hk in range(8):
                nc.sync.dma_start(wout_sb[:, ts(hk, H)], wout[ts(hk, 128), :])
            nc.vector.memset(ones_sb[:, :], 1.0)

            kt_rot = sb.tile([128, 2 * N], BF16)   # [pair pr at pr*N][n]
            qt_rot = sb.tile([128, 2 * N], BF16)
            vt_all = sb.tile([128, KC * LVW], BF16)
            # attn^T laid out as [qc][row-block j][pair][row-in-block] so each
            # AllToAll shard (qc, j) is one contiguous 256-wide span
            attn_sb = sb.tile([128, 2 * N], BF16)
            attn4 = attn_sb.rearrange("p (q j r x) -> p q j r x", q=QC, j=4, r=2)

            def pair_rotary(psums, dst_ap):
                """Whole-pair rotary: stage 4 proj PSUMs to bf16, then 6
                full-width bf16 DVE ops (2x mode, overhead amortized)."""
                stage = sb.tile([128, N], BF16, tag="stg", bufs=2, name="stg")
                for sc, p in enumerate(psums):
                    nc.scalar.copy(stage[:, ts(sc, 512)], p[:, :])
                tmp = sb.tile([128, N], BF16, tag="rota", bufs=2, name="rota")
                tmp2 = sb.tile([128, N], BF16, tag="rotb", bufs=2, name="rotb")
                for hh in (0, 64):
                    nc.vector.tensor_tensor(
                        tmp[hh : hh + 32, :], stage[hh + 32 : hh + 64, :],
                        sinm_sb[hh + 32 : hh + 64, :], MULT)
                    nc.vector.tensor_tensor(
                        tmp[hh + 32 : hh + 64, :], stage[hh : hh + 32, :],
                        sinm_sb[hh : hh + 32, :], MULT)
                nc.vector.tensor_tensor(tmp2[:, :], stage[:, :], cos2_sb[:, :],
                                        MULT)
                nc.vector.tensor_tensor(dst_ap, tmp2[:, :], tmp[:, :], ADD)

            def qk_pair(col0, pr, dst):
                psums = [proj_group(col0, pr, sc) for sc in range(4)]
                pair_rotary(psums, dst[:, pr * N:][:, :N])

            def proj_group(col0, pr, sc):
                p = ps.tile([128, 512], F32, tag="b", name="pp")
                for hk in range(8):
                    nc.tensor.matmul(
                        p[:, :],
                        lhsT=wqk_sb[:, hk * 2 * LQK + col0 + pr * 128:][:, :128],
                        rhs=xt_sb[:, hk * N + sc * 512:][:, :512],
                        start=(hk == 0),
                        stop=(hk == 7),
                    )
                return p

            def q_proj_rot(pr, sc):
                p = proj_group(0, pr, sc)
                rotary(p, qt_rot[:, pr * N + sc * 512:][:, :512], sc)

            # K pair0, Q pair0, V, K pair1, Q pair1 — attention on pair 0
            # can start while pair 1 projects
            qk_pair(LQK, 0, kt_rot)
            qk_pair(0, 0, qt_rot)

            # V projection straight into per-kc tiles. The per-head ones
            # columns are set once upfront; copies use a strided AP that
            # skips them, so nothing in the attention stream waits on DVE.
            nc.vector.memset(
                vt_all.rearrange("p (g e) -> p g e", e=65)[:, :, 64:65], 1.0)
            for rc in range(KC):
                p = ps.tile([128, LVW], F32, tag="b", name="vp")
                for hk in range(8):
                    nc.tensor.matmul(
                        p[:, :],
                        lhsT=xt_sb[:, hk * N + rc * 128:][:, :128],
                        rhs=wv_sb[:, ts(hk, LVW)],
                        start=(hk == 0),
                        stop=(hk == 7),
                    )
                nc.scalar.copy(
                    vt_all[:, ts(rc, LVW)].rearrange(
                        "p (h e) -> p h e", e=65)[:, :, 0:64],
                    p.rearrange("p (h e) -> p h e", e=65)[:, :, 0:64])
            qk_pair(LQK, 1, kt_rot)
            qk_pair(0, 1, qt_rot)

            # ---- attention; per-piece AllToAll + local output projection ----
            def emit_norm(st):
                av0, av1, qc, pr = st
                rd0 = sb.tile([1, 512], BF16, tag="rcd", bufs=4, name="rd0")
                rd1 = sb.tile([1, 512], BF16, tag="rcd", bufs=4, name="rd1")
                nc.vector.tensor_copy(rd0[:, :], av0[64:65, :])
                nc.vector.tensor_copy(rd1[:, :], av1[64:65, :])
                b_ps = ps.tile([128, 512], F32, tag="b", name="b_ps")
                nc.tensor.matmul(b_ps[0:64, :], lhsT=ones_sb[:, :],
                                 rhs=rd0[:, :], start=True, stop=True,
                                 tile_position=(0, 0))
                nc.tensor.matmul(b_ps[64:128, :], lhsT=ones_sb[:, :],
                                 rhs=rd1[:, :], start=True, stop=True,
                                 tile_position=(0, 64))
                bd_sb = sb.tile([128, 512], F32, tag="bsd", bufs=2, name="bd_sb")
                nc.vector.tensor_copy(bd_sb[:, :], b_ps[:, :])
                b_sb = sb.tile([128, 512], F32, tag="bsb", bufs=2, name="b_sb")
                nc.vector.reciprocal_approx_fast(out=b_sb[:, :], in_=bd_sb[:, :])
                dst = attn4[:, qc, :, pr, :]  # [128, 4, 128]
                b3 = b_sb.rearrange("p (j x) -> p j x", x=128)
                nc.vector.tensor_tensor(dst[0:64], av0[0:64, :].rearrange(
                    "p (j x) -> p j x", x=128), b3[0:64], MULT)
                nc.vector.tensor_tensor(dst[64:128], av1[0:64, :].rearrange(
                    "p (j x) -> p j x", x=128), b3[64:128], MULT)

            def emit_a2a(qc):
                # shard j of a2a_in = my 2 head-pair chunks for row block
                # j%4, duplicated to both batch groups (receiver masks off
                # the cross-batch half)
                for j in range(8):
                    nc.sync.dma_start(
                        a2a_in[qc][j].rearrange("(r p) x -> p r x", p=128),
                        attn4[:, qc, j % 4, :, :])
                nc.gpsimd.collective_compute(
                    "AllToAll", BYPASS, replica_groups=[list(range(8))],
                    ins=[a2a_in[qc].opt()], outs=[a2a_out[qc].opt()])

            def emit_outproj(qc):
                # raw slots from all 8 ranks, then mask-combine batch halves
                att_r = sb.tile([128, 16 * 128], BF16, tag="attr", bufs=2,
                                name="att_r")
                r3 = att_r.rearrange("p (c x) -> p c x", x=128)
                for i in range(8):
                    nc.sync.dma_start(
                        r3[:, 2 * i : 2 * i + 2, :],
                        a2a_out[qc][i].rearrange("(c p) x -> p c x", p=128))
                att_g = sb.tile([128, 8 * 128], BF16, tag="attg", bufs=2,
                                name="att_g")
                g3 = att_g.rearrange("p (c x) -> p c x", x=128)
                tmpm = sb.tile([128, 128], BF16, tag="tmpm", bufs=2, name="tmpm")
                for hc in range(8):
                    lo = r3[:, (hc // 2) * 2 + (hc % 2), :]
                    hi = r3[:, 8 + (hc // 2) * 2 + (hc % 2), :]
                    nc.vector.tensor_scalar_mul(tmpm[:, :], hi, msk_sb[:, 1:2])
                    nc.vector.scalar_tensor_tensor(
                        g3[:, hc, :], lo, msk_sb[:, 0:1], tmpm[:, :],
                        MULT, ADD)
                for nh in range(2):
                    o_ps = ps.tile([128, 512], F32, tag="b", name="o_ps")
                    for hc in range(8):
                        nc.tensor.matmul(
                            o_ps[:, :],
                            lhsT=g3[:, hc, :],
                            rhs=wout_sb[:, hc * H + nh * 512:][:, :512],
                            start=(hc == 0),
                            stop=(hc == 7),
                        )
                    ob = sb.tile([128, 512], F32, tag="ob", bufs=3, name="ob")
                    nc.vector.tensor_copy(ob[:, :], o_ps[:, :])
                    nc.sync.dma_start(out[qc, :, ts(nh, 512)], ob[:, :])

            norm_pending = None   # (av0, av1, qc, pr)
            a2a_ready = []        # pieces normalized, awaiting A2A emission
            op_ready = []         # pieces with A2A emitted, awaiting outproj
            for qc in range(QC):
                for pr in range(LPAIRS):
                    qt_p = qt_rot[:, pr * N + qc * 512:][:, :512]
                    av0 = ps.tile([65, 512], F32, tag="av", name="av0")
                    av1 = ps.tile([65, 512], F32, tag="av", name="av1")
                    exps = []
                    for kc in range(KC):
                        s_ps = ps.tile([128, 1024], F32, tag="s", name="s_ps")
                        nc.tensor.matmul(
                            s_ps[:, 0:512],
                            lhsT=kt_rot[0:64, pr * N + kc * 128:][:, :128],
                            rhs=qt_p[0:64, :], start=True, stop=True,
                            tile_position=(0, 0))
                        nc.tensor.matmul(
                            s_ps[:, 512:1024],
                            lhsT=kt_rot[64:128, pr * N + kc * 128:][:, :128],
                            rhs=qt_p[64:128, :], start=True, stop=True,
                            tile_position=(64, 0))
                        e = sb.tile([128, 1024], BF16, tag="exp", bufs=4, name="e")
                        nc.scalar.activation(e[:, :], s_ps[:, :], EXP, scale=0.125)
                        exps.append(e)
                        if kc == 1 and norm_pending is not None:
                            emit_norm(norm_pending)
                            if norm_pending[3] == 1:  # piece complete
                                a2a_ready.append(norm_pending[2])
                            norm_pending = None
                        if kc == 3 and pr == 0 and a2a_ready:
                            emit_a2a(a2a_ready.pop(0))
                        if kc == 8 and qc == 3:
                            emit_outproj(pr)  # pieces 0 and 1
                        if kc > 0:
                            _av_mm(nc, vt_all, exps[kc - 1], av0, av1, kc - 1, pr)
                    _av_mm(nc, vt_all, exps[KC - 1], av0, av1, KC - 1, pr)
                    norm_pending = (av0, av1, qc, pr)
            emit_norm(norm_pending)
            emit_a2a(QC - 1)
            emit_outproj(QC - 2)
            emit_outproj(QC - 1)

    nc.finalize()
    return nc


def _av_mm(nc, vt_all, e, av0, av1, kc, pr):
    nc.tensor.matmul(
        av0[:, :], lhsT=vt_all[:, kc * LVW + 65 * (2 * pr):][:, :65],
        rhs=e[:, 0:512], start=(kc == 0), stop=(kc == KC - 1))
    nc.tensor.matmul(
        av1[:, :], lhsT=vt_all[:, kc * LVW + 65 * (2 * pr + 1):][:, :65],
        rhs=e[:, 512:1024], start=(kc == 0), stop=(kc == KC - 1))


_NC = None


def _get_nc():
    global _NC
    if _NC is None:
        _NC = build_nc()
    return _NC


def _bf16(a):
    return np.ascontiguousarray(a.astype(ml_dtypes.bfloat16))


def make_in_maps(x, rotary_emb, w_qkv, w_out):
    x = np.asarray(x, np.float32)
    rotary_emb = np.asarray(rotary_emb, np.float32)
    w_qkv = np.asarray(w_qkv, np.float32)
    w_out = np.asarray(w_out, np.float32)
    cosT = np.cos(rotary_emb).T.astype(np.float32)  # [64, N]
    sinT = np.sin(rotary_emb).T.astype(np.float32)
    cos2_a = _bf16(np.concatenate([cosT, cosT], axis=0))
    sswp = np.concatenate([sinT[32:], -sinT[:32]], axis=0)
    sinm_a = _bf16(np.concatenate([sswp, sswp], axis=0))
    wout_bf = _bf16(w_out)
    in_maps = []
    for c in range(NC_):
        b, hb = c // 4, c % 4
        h0 = LH * hb
        wq_loc = w_qkv[:, 64 * h0 : 64 * h0 + LQK]
        wk_loc = w_qkv[:, H + 64 * h0 : H + 64 * h0 + LQK]
        wv_loc = w_qkv[:, 2 * H + 64 * h0 : 2 * H + 64 * h0 + LQK]
        wv_aug = np.zeros((H, LVW), np.float32)
        for j in range(LH):
            wv_aug[:, 65 * j : 65 * j + 64] = wv_loc[:, 64 * j : 64 * j + 64]
        msk_a = np.zeros((128, 2), np.float32)
        msk_a[:, b] = 1.0
        in_maps.append({
            "xT": _bf16(x[b].T),
            "msk": msk_a,
            "wqk": _bf16(np.concatenate([wq_loc, wk_loc], axis=1)),
            "wv": _bf16(wv_aug),
            "wout": wout_bf,
            "cos2": cos2_a,
            "sinm": sinm_a,
        })
    return in_maps


def run(x, rotary_emb, w_qkv, w_out, trace=False, tmpdir=None):
    nc = _get_nc()
    in_maps = make_in_maps(x, rotary_emb, w_qkv, w_out)
    res = run_bass_kernel_spmd(nc, in_maps, list(range(NC_)), trace=trace,
                               tmpdir=tmpdir)
    full = np.empty((B, N, H), np.float32)
    for c in range(NC_):
        b, r = c // 4, c % 4
        piece = np.asarray(res.results[c]["out"], np.float32)  # [QC, 128, H]
        for qc in range(QC):
            full[b, 512 * qc + 128 * r : 512 * qc + 128 * r + 128] = piece[qc]
    return full, res


def kernel(x, rotary_emb, w_qkv, w_out):
    full, _ = run(x, rotary_emb, w_qkv, w_out)
    return full



# revision 7
# speedup vs baseline: 1.0492x; 1.0492x over previous
"""Trainium2 Bass kernel for fused multi-head attention (16 heads, d=64,
b=2, n=2048, h=1024) across 8 NeuronCores.

Sharding: tensor-parallel over heads x data-parallel over batch.
Core c handles batch c//4 and heads [4*(c%4), 4*(c%4)+4). After
attention, a bf16 AllToAll per 512-row piece (Ulysses-style) swaps
head-shards for row-shards so each core runs the output projection
locally over the full 1024-dim contraction.

v2 schedule: one software-pipelined stream. Q/K projections are emitted
just-in-time in 1024-column chunks; the V projection and pair-1
projections are interleaved into the attention matmul stream, so ScalarE
exp (the pace-setting engine, ~1.1us per [128,1024] tile) starts ~25us
into the kernel instead of after a serial 60us projection phase.
Attention positions run in order (pr,qc) = (0,0),(0,1),(1,0),(0,2),
(1,1),(0,3),(1,2),(1,3) so AllToAll pieces complete early and their
~20us collective latency hides under later positions; output projections
for pieces 0/1 are interleaved into positions 5/7 and pieces 2/3 overlap
the tail collective. Per-position attn_out^T (+ denominator row from a
ones-column in V) is copied out of PSUM immediately after the last AV so
the next position's accumulation never waits; the normalization chain is
deferred to the next position's kc=1 slot. The AllToAll batch-half
combine is two full-width DVE ops. ScalarE runs exp exclusively; staging
copies are on VectorE.
"""

import sys

if "/opt/trn_rl_repo" not in sys.path:
    sys.path.insert(0, "/opt/trn_rl_repo")

import numpy as np
import ml_dtypes

import concourse.bass as bass
import concourse.mybir as mybir
import concourse.tile as tile
from concourse import bacc
from concourse.bass import ts
from concourse.bass_utils import run_bass_kernel_spmd

BF16 = mybir.dt.bfloat16
F32 = mybir.dt.float32
ADD = mybir.AluOpType.add
MULT = mybir.AluOpType.mult
BYPASS = mybir.AluOpType.bypass
EXP = mybir.ActivationFunctionType.Exp

HEADS, D, H, N, B = 16, 64, 1024, 2048, 2
NC_ = 8
LH = 4            # local heads per core
LPAIRS = 2        # local head pairs
KC = 16           # k chunks of 128 over n=2048
QC = 4            # q chunks of 512 over n=2048 (= AllToAll pieces)
LVW = LH * 65     # 260: local v-aug width
LQK = LH * D      # 256 local q (or k) columns


def build_nc():
    nc = bacc.Bacc("TRN2", target_bir_lowering=False, debug=False, num_devices=NC_)

    xT = nc.declare_dram_parameter("xT", [H, N], BF16, isOutput=False)
    wqk = nc.declare_dram_parameter("wqk", [H, 2 * LQK], BF16, isOutput=False)
    wv = nc.declare_dram_parameter("wv", [H, LVW], BF16, isOutput=False)
    wout = nc.declare_dram_parameter("wout", [H, H], BF16, isOutput=False)
    cos2 = nc.declare_dram_parameter("cos2", [128, N], BF16, isOutput=False)
    # sinm[p] = sin value read at SOURCE partition p during the shuffle:
    # p%64 < 32 -> +sin[p%64+32], else -sin[p%64-32]
    sinm = nc.declare_dram_parameter("sinm", [128, N], BF16, isOutput=False)
    # msk[:,0]=1 iff this core's batch is 0; msk[:,1]=1 iff batch 1
    msk = nc.declare_dram_parameter("msk", [128, 2], F32, isOutput=False)
    out = nc.declare_dram_parameter("out", [QC, 128, H], F32, isOutput=True)

    with tile.TileContext(nc) as tc:
        with (
            tc.tile_pool(name="dram", bufs=1, space="DRAM") as dram,
            tc.tile_pool(name="sb", bufs=1) as sb,
            tc.tile_pool(name="sbw", bufs=1) as sbw,
            tc.tile_pool(name="psum", bufs=2, space="PSUM") as ps,
        ):
            a2a_in = [dram.tile([8, 2 * 128, 128], BF16, name=f"ain{i}")
                      for i in range(QC)]
            a2a_out = [dram.tile([8, 2 * 128, 128], BF16, name=f"aout{i}")
                       for i in range(QC)]

            # warmup collective first: absorbs the one-time CC barrier
            # (~40us) under the staging/projection prologue
            warm_in = dram.tile([8, 128], BF16, name="warm_in")
            warm_out = dram.tile([8, 128], BF16, name="warm_out")
            warm_sb = sbw.tile([1, 128], BF16)
            nc.vector.memset(warm_sb[:, :], 0.0)
            nc.scalar.dma_start(warm_in[0:1, :], warm_sb[:, :])
            nc.gpsimd.collective_compute(
                "AllToAll", BYPASS, replica_groups=[list(range(8))],
                ins=[warm_in.opt()], outs=[warm_out.opt()])

            # ---- stage inputs: wqk + x first (4 queues); wout deferred ----
            xt_sb = sbw.tile([128, 8 * N], BF16)
            wqk_sb = sbw.tile([128, 8 * 2 * LQK], BF16)
            wv_sb = sbw.tile([128, 8 * LVW], BF16)
            wout_sb = sbw.tile([128, 8 * H], BF16)
            cos2_sb = sbw.tile([128, N], BF16)
            sinm_sb = sbw.tile([128, N], BF16)
            ones_sb = sbw.tile([1, D], BF16)
            msk_sb = sbw.tile([128, 2], F32)
            for hk in range(8):
                nc.sync.dma_start(wqk_sb[:, ts(hk, 2 * LQK)], wqk[ts(hk, 128), :])
            engs = [nc.scalar, nc.gpsimd, nc.sync]
            for hk in range(8):
                engs[hk % 3].dma_start(xt_sb[:, ts(hk, N)], xT[ts(hk, 128), :])
            nc.gpsimd.dma_start(cos2_sb[:, :], cos2[:, :])
            nc.gpsimd.dma_start(sinm_sb[:, :], sinm[:, :])
            for hk in range(8):
                nc.scalar.dma_start(wv_sb[:, ts(hk, LVW)], wv[ts(hk, 128), :])
            nc.gpsimd.dma_start(msk_sb[:, :], msk[:, :])
            nc.vector.memset(ones_sb[:, :], 1.0)

            kt_rot = sb.tile([128, 2 * N], BF16)   # [pair pr at pr*N][n]
            qt_rot = sb.tile([128, 2 * N], BF16)
            vt_all = sb.tile([128, KC * LVW], BF16)
            # attn^T laid out as [qc][row-block j][pair][row-in-block] so each
            # AllToAll shard (qc, j) is one contiguous 256-wide span
            attn_sb = sb.tile([128, 2 * N], BF16)
            attn4 = attn_sb.rearrange("p (q j r x) -> p q j r x", q=QC, j=4, r=2)

            # per-head ones columns of v-aug, set once
            nc.vector.memset(
                vt_all.rearrange("p (g e) -> p g e", e=65)[:, :, 64:65], 1.0)

            def proj_group(col0, sc):
                p = ps.tile([128, 512], F32, tag="b", name="pp")
                for hk in range(8):
                    nc.tensor.matmul(
                        p[:, :],
                        lhsT=wqk_sb[:, hk * 2 * LQK + col0:][:, :128],
                        rhs=xt_sb[:, hk * N + sc * 512:][:, :512],
                        start=(hk == 0),
                        stop=(hk == 7),
                    )
                return p

            def rotary_apply(psums, dst, pos0):
                """Rotary on a [128, 1024] chunk covering positions
                [pos0, pos0+1024): stage psums to bf16 (DVE), then the
                partition-swapped sin multiply + cos multiply + add."""
                W = 1024
                stage = sb.tile([128, W], BF16, tag="stg", bufs=2, name="stg")
                for i, p in enumerate(psums):
                    nc.vector.tensor_copy(stage[:, ts(i, 512)], p[:, :])
                tmp = sb.tile([128, W], BF16, tag="rta", bufs=2, name="rta")
                tmp2 = sb.tile([128, W], BF16, tag="rtb", bufs=2, name="rtb")
                sl = sinm_sb[:, pos0:pos0 + W]
                cl = cos2_sb[:, pos0:pos0 + W]
                for hh in (0, 64):
                    nc.vector.tensor_tensor(
                        tmp[hh:hh + 32, :], stage[hh + 32:hh + 64, :],
                        sl[hh + 32:hh + 64, :], MULT)
                    nc.vector.tensor_tensor(
                        tmp[hh + 32:hh + 64, :], stage[hh:hh + 32, :],
                        sl[hh:hh + 32, :], MULT)
                nc.vector.tensor_tensor(tmp2[:, :], stage[:, :], cl, MULT)
                nc.vector.tensor_tensor(dst, tmp2[:, :], tmp[:, :], ADD)

            def v_chunk(rc):
                p = ps.tile([128, LVW], F32, tag="b", name="vp")
                for hk in range(8):
                    nc.tensor.matmul(
                        p[:, :],
                        lhsT=xt_sb[:, hk * N + rc * 128:][:, :128],
                        rhs=wv_sb[:, ts(hk, LVW)],
                        start=(hk == 0),
                        stop=(hk == 7),
                    )
                nc.vector.tensor_copy(
                    vt_all[:, ts(rc, LVW)].rearrange(
                        "p (h e) -> p h e", e=65)[:, :, 0:64],
                    p.rearrange("p (h e) -> p h e", e=65)[:, :, 0:64])

            def _av_mm(e, av0, av1, kc, pr):
                nc.tensor.matmul(
                    av0[:, :], lhsT=vt_all[:, kc * LVW + 65 * (2 * pr):][:, :65],
                    rhs=e[:, 0:512], start=(kc == 0), stop=(kc == KC - 1))
                nc.tensor.matmul(
                    av1[:, :],
                    lhsT=vt_all[:, kc * LVW + 65 * (2 * pr + 1):][:, :65],
                    rhs=e[:, 512:1024], start=(kc == 0), stop=(kc == KC - 1))

            def finish_copy(av0, av1):
                # move attn_out^T (+denominator row 64) out of PSUM right
                # away so the next position's AV accumulation can reuse the
                # banks; the normalization chain runs later from SBUF
                a0 = sb.tile([65, 512], BF16, tag="avs", bufs=4, name="a0")
                a1 = sb.tile([65, 512], BF16, tag="avs", bufs=4, name="a1")
                nc.vector.tensor_copy(a0[:, :], av0[:, :])
                nc.vector.tensor_copy(a1[:, :], av1[:, :])
                return a0, a1

            def finish_norm(qc, pr, a0, a1):
                # denominator rows to partition 0 (matmul operands must share
                # a base partition)
                ad = sb.tile([1, 1024], BF16, tag="adn", bufs=2, name="ad")
                nc.vector.tensor_copy(ad[:, 0:512], a0[64:65, :])
                nc.vector.tensor_copy(ad[:, 512:1024], a1[64:65, :])
                b_ps = ps.tile([128, 512], F32, tag="b", name="b_ps")
                nc.tensor.matmul(b_ps[0:64, :], lhsT=ones_sb[:, :],
                                 rhs=ad[:, 0:512], start=True, stop=True,
                                 tile_position=(0, 0))
                nc.tensor.matmul(b_ps[64:128, :], lhsT=ones_sb[:, :],
                                 rhs=ad[:, 512:1024], start=True, stop=True,
                                 tile_position=(0, 64))
                bd_sb = sb.tile([128, 512], F32, tag="bsd", bufs=2, name="bd_sb")
                nc.vector.tensor_copy(bd_sb[:, :], b_ps[:, :])
                b_sb = sb.tile([128, 512], F32, tag="bsb", bufs=2, name="b_sb")
                nc.vector.reciprocal_approx_fast(out=b_sb[:, :], in_=bd_sb[:, :])
                # both TT inputs must share a base partition: bring head B's
                # inv-denominators down to partitions 0-63
                b_lo = sb.tile([64, 512], F32, tag="blo", bufs=2, name="b_lo")
                nc.vector.tensor_copy(b_lo[:, :], b_sb[64:128, :])
                dst = attn4[:, qc, :, pr, :]  # [128, 4, 128]
                b3 = b_sb.rearrange("p (j x) -> p j x", x=128)
                bl3 = b_lo.rearrange("p (j x) -> p j x", x=128)
                nc.vector.tensor_tensor(dst[0:64], a0[0:64, :].rearrange(
                    "p (j x) -> p j x", x=128), b3[0:64], MULT)
                nc.vector.tensor_tensor(dst[64:128], a1[0:64, :].rearrange(
                    "p (j x) -> p j x", x=128), bl3[:, :, :], MULT)

            def a2a_send(qc, r):
                # shard j = my head-pair chunk(s) for row block j%4,
                # duplicated to both batch groups (receiver masks off the
                # cross-batch half); r=None sends both pairs
                for j in range(8):
                    d = a2a_in[qc][j].rearrange("(r p) x -> p r x", p=128)
                    if r is None:
                        nc.sync.dma_start(d, attn4[:, qc, j % 4, :, :])
                    else:
                        nc.sync.dma_start(d[:, r:r + 1, :],
                                          attn4[:, qc, j % 4, r:r + 1, :])

            def a2a_go(qc):
                nc.gpsimd.collective_compute(
                    "AllToAll", BYPASS, replica_groups=[list(range(8))],
                    ins=[a2a_in[qc].opt()], outs=[a2a_out[qc].opt()])

            def emit_a2a(qc):
                a2a_send(qc, None)
                a2a_go(qc)

            # outproj split into stages so its TensorE work spreads over a
            # later attention position
            def outproj_recv(qc):
                att_r = sb.tile([128, 16 * 128], BF16, tag="attr", bufs=2,
                                name="att_r")
                r3 = att_r.rearrange("p (c x) -> p c x", x=128)
                for i in range(8):
                    nc.sync.dma_start(
                        r3[:, 2 * i: 2 * i + 2, :],
                        a2a_out[qc][i].rearrange("(c p) x -> p c x", p=128))
                # full-width mask-combine of the two batch halves
                att_g = sb.tile([128, 8 * 128], BF16, tag="attg", bufs=2,
                                name="att_g")
                tmpm = sb.tile([128, 8 * 128], BF16, tag="tmpm", bufs=2,
                               name="tmpm")
                nc.vector.tensor_scalar_mul(
                    tmpm[:, :], att_r[:, 1024:2048], msk_sb[:, 1:2])
                nc.vector.scalar_tensor_tensor(
                    att_g[:, :], att_r[:, 0:1024], msk_sb[:, 0:1], tmpm[:, :],
                    MULT, ADD)
                return att_g

            def outproj_mm(qc, att_g, nh):
                g3 = att_g.rearrange("p (c x) -> p c x", x=128)
                o_ps = ps.tile([128, 512], F32, tag="b", name="o_ps")
                for hc in range(8):
                    nc.tensor.matmul(
                        o_ps[:, :],
                        lhsT=g3[:, hc, :],
                        rhs=wout_sb[:, hc * H + nh * 512:][:, :512],
                        start=(hc == 0),
                        stop=(hc == 7),
                    )
                ob = sb.tile([128, 512], F32, tag="ob", bufs=3, name="ob")
                nc.vector.tensor_copy(ob[:, :], o_ps[:, :])
                nc.sync.dma_start(out[qc, :, ts(nh, 512)], ob[:, :])

            def att_pos(qc, pr, hooks):
                qt_p = qt_rot[:, pr * N + qc * 512:][:, :512]
                av0 = ps.tile([65, 512], F32, tag="av", name="av0")
                av1 = ps.tile([65, 512], F32, tag="av", name="av1")
                exps = []
                for kc in range(KC):
                    s_ps = ps.tile([128, 1024], F32, tag="s", name="s_ps")
                    nc.tensor.matmul(
                        s_ps[:, 0:512],
                        lhsT=kt_rot[0:64, pr * N + kc * 128:][:, :128],
                        rhs=qt_p[0:64, :], start=True, stop=True,
                        tile_position=(0, 0))
                    nc.tensor.matmul(
                        s_ps[:, 512:1024],
                        lhsT=kt_rot[64:128, pr * N + kc * 128:][:, :128],
                        rhs=qt_p[64:128, :], start=True, stop=True,
                        tile_position=(64, 0))
                    e = sb.tile([128, 1024], BF16, tag="exp", bufs=4, name="e")
                    nc.scalar.activation(e[:, :], s_ps[:, :], EXP, scale=0.125)
                    exps.append(e)
                    for f in hooks.get(kc, []):
                        f()
                    if kc > 0:
                        _av_mm(exps[kc - 1], av0, av1, kc - 1, pr)
                _av_mm(exps[KC - 1], av0, av1, KC - 1, pr)
                return finish_copy(av0, av1)

            # ---- prologue: K pair 0 (both halves), Q pair 0 low half ----
            def qk_half(col0, dst_tile, half, pos_ofs):
                psums = [proj_group(col0, 2 * half + i) for i in range(2)]
                rotary_apply(
                    psums, dst_tile[:, pos_ofs + half * 1024:][:, :1024],
                    half * 1024)

            qk_half(LQK, kt_rot, 0, 0)
            qk_half(LQK, kt_rot, 1, 0)
            qk_half(0, qt_rot, 0, 0)
            # pre-fill 4 V chunks so TensorE has ready work while the first
            # rotary chains run on DVE
            for rc in range(4):
                v_chunk(rc)

            # closure helpers for hook tables
            def mk(f, *a):
                return lambda: f(*a)

            grabs = {}

            def grab(key, col0, sc):
                def g():
                    grabs.setdefault(key, []).append(proj_group(col0, sc))
                return g

            def rot(key, dst_tile, pos_ofs, half):
                def g():
                    rotary_apply(
                        grabs.pop(key),
                        dst_tile[:, pos_ofs + half * 1024:][:, :1024],
                        half * 1024)
                return g

            pend = {}

            def norm(qc, pr):
                def g():
                    a0, a1 = pend.pop((qc, pr))
                    finish_norm(qc, pr, a0, a1)
                return g

            def wout_dma():
                for hk in range(8):
                    nc.sync.dma_start(wout_sb[:, ts(hk, H)], wout[ts(hk, 128), :])

            op_ag = {}

            def op_recv(qc):
                def g():
                    op_ag[qc] = outproj_recv(qc)
                return g

            def op_mm(qc, nh):
                return lambda: outproj_mm(qc, op_ag[qc], nh)

            KQ1, QQ1 = LQK + 128, 128
            SEQ = [
                # (pr, qc, hooks); norm(qc, pr) finalizes the PREVIOUS position
                (0, 0, {kc: [mk(v_chunk, kc + 4)] for kc in range(12)}),
                (0, 1, {
                    1: [norm(0, 0)],
                    2: [wout_dma, grab("k1a", KQ1, 0)],
                    4: [grab("k1a", KQ1, 1)],
                    6: [rot("k1a", kt_rot, N, 0)],
                    7: [grab("q1a", QQ1, 0)],
                    9: [grab("q1a", QQ1, 1)],
                    11: [rot("q1a", qt_rot, N, 0)],
                    12: [grab("k1b", KQ1, 2)],
                    14: [grab("k1b", KQ1, 3)],
                }),
                (1, 0, {
                    0: [rot("k1b", kt_rot, N, 1)],
                    1: [norm(1, 0)],
                    2: [grab("q0b", 0, 2)],
                    4: [grab("q0b", 0, 3)],
                    6: [rot("q0b", qt_rot, 0, 1)],
                }),
                (0, 2, {
                    1: [norm(0, 1)],
                    2: [grab("q1b", QQ1, 2)],
                    4: [grab("q1b", QQ1, 3), mk(emit_a2a, 0)],
                    6: [rot("q1b", qt_rot, N, 1)],
                }),
                (1, 1, {
                    1: [norm(2, 0)],
                    6: [op_recv(0)],
                    8: [op_mm(0, 0)],
                    12: [op_mm(0, 1)],
                }),
                (0, 3, {
                    1: [norm(1, 1)],
                    4: [mk(emit_a2a, 1)],
                }),
                (1, 2, {
                    1: [norm(3, 0)],
                    8: [op_recv(1)],
                    10: [op_mm(1, 0)],
                    13: [op_mm(1, 1)],
                }),
                (1, 3, {
                    1: [norm(2, 1)],
                    4: [mk(emit_a2a, 2)],
                    8: [mk(a2a_send, 3, 0)],
                }),
            ]
            for pr, qc, hooks in SEQ:
                pend[(qc, pr)] = att_pos(qc, pr, hooks)

            # tail: piece 2 outproj overlaps the piece 3 collective
            norm(3, 1)()
            a2a_send(3, 1)
            a2a_go(3)
            ag2 = outproj_recv(2)
            outproj_mm(2, ag2, 0)
            outproj_mm(2, ag2, 1)
            ag3 = outproj_recv(3)
            outproj_mm(3, ag3, 0)
            outproj_mm(3, ag3, 1)

    nc.finalize()
    return nc


_NC = None


def _get_nc():
    global _NC
    if _NC is None:
        _NC = build_nc()
    return _NC


def _bf16(a):
    return np.ascontiguousarray(a.astype(ml_dtypes.bfloat16))


def make_in_maps(x, rotary_emb, w_qkv, w_out):
    x = np.asarray(x, np.float32)
    rotary_emb = np.asarray(rotary_emb, np.float32)
    w_qkv = np.asarray(w_qkv, np.float32)
    w_out = np.asarray(w_out, np.float32)
    cosT = np.cos(rotary_emb).T.astype(np.float32)  # [64, N]
    sinT = np.sin(rotary_emb).T.astype(np.float32)
    cos2_a = _bf16(np.concatenate([cosT, cosT], axis=0))
    sswp = np.concatenate([sinT[32:], -sinT[:32]], axis=0)
    sinm_a = _bf16(np.concatenate([sswp, sswp], axis=0))
    wout_bf = _bf16(w_out)
    in_maps = []
    for c in range(NC_):
        b, hb = c // 4, c % 4
        h0 = LH * hb
        wq_loc = w_qkv[:, 64 * h0: 64 * h0 + LQK]
        wk_loc = w_qkv[:, H + 64 * h0: H + 64 * h0 + LQK]
        wv_loc = w_qkv[:, 2 * H + 64 * h0: 2 * H + 64 * h0 + LQK]
        wv_aug = np.zeros((H, LVW), np.float32)
        for j in range(LH):
            wv_aug[:, 65 * j: 65 * j + 64] = wv_loc[:, 64 * j: 64 * j + 64]
        msk_a = np.zeros((128, 2), np.float32)
        msk_a[:, b] = 1.0
        in_maps.append({
            "xT": _bf16(x[b].T),
            "msk": msk_a,
            "wqk": _bf16(np.concatenate([wq_loc, wk_loc], axis=1)),
            "wv": _bf16(wv_aug),
            "wout": wout_bf,
            "cos2": cos2_a,
            "sinm": sinm_a,
        })
    return in_maps


def run(x, rotary_emb, w_qkv, w_out, trace=False, tmpdir=None):
    nc = _get_nc()
    in_maps = make_in_maps(x, rotary_emb, w_qkv, w_out)
    res = run_bass_kernel_spmd(nc, in_maps, list(range(NC_)), trace=trace,
                               tmpdir=tmpdir)
    full = np.empty((B, N, H), np.float32)
    for c in range(NC_):
        b, r = c // 4, c % 4
        piece = np.asarray(res.results[c]["out"], np.float32)  # [QC, 128, H]
        for qc in range(QC):
            full[b, 512 * qc + 128 * r: 512 * qc + 128 * r + 128] = piece[qc]
    return full, res


def kernel(x, rotary_emb, w_qkv, w_out):
    full, _ = run(x, rotary_emb, w_qkv, w_out)
    return full
